# revision 1
# baseline (speedup 1.0000x reference)
"""Trainium2 Bass kernel for nn_DividedModel (64 independent MLP towers).

Math (per tower o of O=64):
    h0 = relu(x @ W0[o] + b0[o])         x: [B, 32], W0[o]: [32, 64]
    h1 = relu(h0 @ W1[o] + b1[o])        W1[o]: [64, 64]
    h2 = relu(h1 @ W2[o] + b2[o])        W2[o]: [64, 64]
    out[:, o] = h2 @ W3[o] + b3[o]       W3[o]: [64]

Strategy:
  - Data-parallel: batch B=16384 sharded 8 ways (2048 rows/core), params
    replicated on every core; no collectives.
  - Activations are kept transposed ([h, batch]) so weights are the
    stationary matmul operand and no transposes are ever needed; the
    input x is transposed on the host, the output is produced as
    [O, B_local] and transposed back on the host.
  - Matmuls run in float32r (TF32-like, ~1.5e-4 per-op relative error),
    which streams at 1 row/cycle on the TRN2 PE vs 4 cycles/row for
    exact fp32 - a 4x matmul speedup for ~3e-4 end-to-end rel err.
  - Tower PAIRS are packed block-diagonally into single 128-wide matmuls:
    lhsT [128, 128] = diag(W[a], W[b]), rhs [128, 512] = [h_a; h_b].
  - b0 is folded into the L0 matmul via an extra ones-row of the input
    (contraction K=66: 32 x-rows + 1 ones-row per tower of the pair);
    b3 is applied by the output copy's per-partition bias; b1/b2 (zero
    in this problem) switch the build to per-bank evacuations with
    per-partition bias APs.
  - L3 is one accumulating matmul chain per batch chunk: 32 block
    matmuls (one per pair) summing into a single [64, 512] PSUM bank.
  - The whole rep is emitted as one software-pipelined loop over 128
    global lanes (chunk x pair) with a 6-slot skew between layers, so
    every engine's program order interleaves many independent lanes.
  - PSUM->SBUF relu evacuation rides ScalarE + VectorE concurrently
    (nc.any lets the Tile scheduler pick the idle engine); in the
    zero-bias build two lanes share a 2-bank PSUM tile so each
    evacuation op covers [128, 1024], amortizing the per-op overhead.

Measured (8 axon-attached NC-v3 cores): ~132-139 us/iteration steady
state, ~3.1e-4 relative error vs the fp32 jax reference. This is at the
per-core PE floor: 512 f32r matmuls x (512 streamed rows @2.4 GHz +
~50-65 ns inline weight load) ~= 137 us; the ScalarE+VectorE PSUM
evacuation floor (~105 us) and all DMA are hidden behind it.
"""

import numpy as np

B, I, O, H = 16384, 32, 64, 64
NCORES = 8
BL = B // NCORES      # 2048 batch rows per core
NB = 512              # batch columns per matmul (one PSUM bank fp32)
CHUNKS = BL // NB     # 4
NPAIR = O // 2        # 32 tower pairs
NGROUP = O // 4       # 16 groups of 2 pairs
SKEW = 6              # software-pipeline slot skew between layers
SPLIT_EVERY = 0       # 0 = off: split every Nth evac into ACT+DVE halves
MM_DT = "f32r"        # matmul operand dtype: f32r | fp16 | f32

_CACHE = {}


def _build(with_bias: bool = False, reps: int = 1, mm_dt: str = MM_DT):
    import concourse.mybir as mybir
    import concourse.tile as tile
    from concourse import bacc

    f32 = mybir.dt.float32
    fp16_via_u16 = mm_dt == "fp16"
    f32r = (mybir.dt.uint16 if fp16_via_u16
            else {'f32r': mybir.dt.float32r, 'f32': f32}[mm_dt])

    def R(ap):
        # In fp16 mode tensors are declared uint16 end-to-end (the axon
        # PJRT bridge rejects F16 transfers); bitcast to f16 at each use.
        return ap.bitcast(mybir.dt.float16) if fp16_via_u16 else ap
    Relu = mybir.ActivationFunctionType.Relu
    Ident = mybir.ActivationFunctionType.Identity
    add_op = mybir.AluOpType.add
    max_op = mybir.AluOpType.max

    nc = bacc.Bacc(None, target_bir_lowering=False, debug=False)

    x2_d = nc.dram_tensor("x2", [66, BL], f32r, kind="ExternalInput")
    wl0_d = nc.dram_tensor("wl0", [66, 128 * NPAIR], f32r, kind="ExternalInput")
    wl1_d = nc.dram_tensor("wl1", [128, 128 * NPAIR], f32r, kind="ExternalInput")
    wl2_d = nc.dram_tensor("wl2", [128, 128 * NPAIR], f32r, kind="ExternalInput")
    wl3_d = nc.dram_tensor("wl3", [128, 64 * NPAIR], f32r, kind="ExternalInput")
    bb_d = nc.dram_tensor("bb", [128, 65], f32, kind="ExternalInput")
    outT_d = nc.dram_tensor("outT", [O, BL], f32, kind="ExternalOutput")

    with tile.TileContext(nc) as tc:
        with (
            tc.tile_pool(name="w", bufs=1) as wpool,
            tc.tile_pool(name="h", bufs=24) as hpool,
            tc.tile_pool(name="ot", bufs=3) as opool,
            tc.tile_pool(name="pp", bufs=(7 if with_bias else 3), space="PSUM") as ppool,
            tc.tile_pool(name="l3", bufs=2, space="PSUM") as l3pool,
        ):
            x2_s = wpool.tile([128, BL], f32r, tag="x2")
            wl0_s = wpool.tile([128, 128 * NPAIR], f32r, tag="wl0")
            wl1_s = wpool.tile([128, 128 * NPAIR], f32r, tag="wl1")
            wl2_s = wpool.tile([128, 128 * NPAIR], f32r, tag="wl2")
            wl3_s = wpool.tile([128, 64 * NPAIR], f32r, tag="wl3")
            bb_s = wpool.tile([128, 65], f32, tag="bb")

            # Spread input loads across the three DMA-capable queues
            # (SP/sync, GpSimd, ScalarE), split and ordered by first use:
            # lane t consumes wl<l> block t at slot t + l*SKEW, so earlier
            # blocks must land first.
            q0 = 32 * NPAIR
            nc.sync.dma_start(bb_s[:], bb_d[:])
            nc.gpsimd.dma_start(wl0_s[0:66, :q0], wl0_d[:, :q0])
            nc.sync.dma_start(x2_s[0:66, :NB], x2_d[:, :NB])
            nc.scalar.dma_start(wl0_s[0:66, q0 : 2 * q0], wl0_d[:, q0 : 2 * q0])
            nc.gpsimd.dma_start(wl1_s[:, :q0], wl1_d[:, :q0])
            nc.scalar.dma_start(wl0_s[0:66, 2 * q0 :], wl0_d[:, 2 * q0 :])
            nc.gpsimd.dma_start(wl2_s[:, :q0], wl2_d[:, :q0])
            nc.scalar.dma_start(wl1_s[:, q0 : 2 * q0], wl1_d[:, q0 : 2 * q0])
            nc.gpsimd.dma_start(wl1_s[:, 2 * q0 :], wl1_d[:, 2 * q0 :])
            nc.scalar.dma_start(wl2_s[:, q0 : 2 * q0], wl2_d[:, q0 : 2 * q0])
            nc.sync.dma_start(x2_s[0:66, NB:], x2_d[:, NB:])
            nc.gpsimd.dma_start(wl2_s[:, 2 * q0 :], wl2_d[:, 2 * q0 :])
            nc.sync.dma_start(wl3_s[:, :q0], wl3_d[:, :q0])
            nc.scalar.dma_start(wl3_s[:, q0:], wl3_d[:, q0:])

            evac_state = [0]

            def evac_relu(dst, src, bias_col):
                """dst[SBUF] = relu(src[PSUM] + bias).

                nc.any: the Tile scheduler routes each op to whichever of
                ScalarE/VectorE is free (GpSimd cannot read PSUM)."""
                i = evac_state[0]
                evac_state[0] += 1
                fd = dst.shape[-1]
                split = SPLIT_EVERY and i % SPLIT_EVERY == 0 and fd > NB
                if bias_col is None:
                    if split:
                        h = fd // 2
                        nc.scalar.activation(dst[:, :h], src[:, :h], Relu)
                        nc.vector.tensor_scalar_max(dst[:, h:], src[:, h:], 0.0)
                    else:
                        nc.any.tensor_scalar_max(R(dst), src, 0.0)
                else:
                    bias_ap = bb_s[:, bias_col : bias_col + 1]
                    nc.any.tensor_scalar(R(dst), src, bias_ap, 0.0, add_op, max_op)

            NLANES = CHUNKS * NPAIR  # 128 global lanes: (chunk, pair)

            def emit_mm(layer, ln, pp, pcol, h_in):
                c, t = divmod(ln, NPAIR)
                if layer == 0:
                    lhsT = wl0_s[0:66, 128 * t : 128 * (t + 1)]
                    rhs = x2_s[0:66, NB * c : NB * (c + 1)]
                elif layer == 1:
                    lhsT = wl1_s[:, 128 * t : 128 * (t + 1)]
                    rhs = h_in
                else:
                    lhsT = wl2_s[:, 128 * t : 128 * (t + 1)]
                    rhs = h_in
                nc.tensor.matmul(
                    pp[:, NB * pcol : NB * (pcol + 1)],
                    R(lhsT),
                    R(rhs),
                    start=True,
                    stop=True,
                )

            for _rep in range(reps):
                l3ps = [None] * CHUNKS
                # per-layer state: current psum tile + h tile (paired lanes
                # share one psum/h tile in the fast zero-bias FD=1024 path)
                W = 1 if with_bias else 2  # lanes per psum/evac tile
                pps = [None, None, None]
                hs = [[None] * NLANES, [None] * NLANES, [None] * NLANES]
                for s in range(NLANES + 3 * SKEW):
                    for layer in range(3):
                        ln = s - layer * SKEW
                        if not (0 <= ln < NLANES):
                            continue
                        c, t = divmod(ln, NPAIR)
                        pcol = ln % W
                        if pcol == 0:
                            pps[layer] = ppool.tile(
                                [128, W * NB], f32, tag="pp", name=f"pp{layer}"
                            )
                            hs[layer][ln] = hpool.tile(
                                [128, W * NB], f32r, tag="h", name=f"h{layer}"
                            )
                        else:
                            hs[layer][ln] = hs[layer][ln - 1]
                        if layer == 0 and t == 0:
                            l3ps[c] = l3pool.tile(
                                [128, NB], f32, tag="l3", name="l3p"
                            )
                        h_in = None
                        if layer > 0:
                            prev = hs[layer - 1][ln]
                            off = (ln % W) * NB if W == 2 else 0
                            h_in = prev[:, off : off + NB]
                        emit_mm(layer, ln, pps[layer], pcol, h_in)
                        if pcol == W - 1:
                            # evacuate the full psum tile in one op
                            dst = hs[layer][ln]
                            if with_bias:
                                bias_col = None if layer == 0 else (
                                    (1 if layer == 1 else 33) + t
                                )
                                evac_relu(dst[:], pps[layer][:], bias_col)
                            else:
                                evac_relu(dst[:], pps[layer][:], None)
                            pps[layer] = None
                    ln = s - 3 * SKEW
                    if 0 <= ln < NLANES:
                        c, t = divmod(ln, NPAIR)
                        off = (ln % W) * NB if W == 2 else 0
                        nc.tensor.matmul(
                            l3ps[c][0:64, :],
                            R(wl3_s[:, 64 * t : 64 * (t + 1)]),
                            R(hs[2][ln][:, off : off + NB]),
                            start=(t == 0),
                            stop=(t == NPAIR - 1),
                        )
                        hs[2][ln] = None
                        if t == NPAIR - 1:
                            out_sb = opool.tile([64, NB], f32, tag="ot")
                            nc.scalar.activation(
                                out_sb[:], l3ps[c][0:64, :], Ident,
                                bias=bb_s[0:64, 0:1],
                            )
                            l3ps[c] = None
                            nc.sync.dma_start(
                                outT_d[:, NB * c : NB * (c + 1)], out_sb[:]
                            )

    nc.compile()
    return nc


def _prep_weights(W0, b0, W1, b1, W2, b2, W3, b3):
    WL0 = np.zeros((66, 128 * NPAIR), np.float32)
    WL1 = np.zeros((128, 128 * NPAIR), np.float32)
    WL2 = np.zeros((128, 128 * NPAIR), np.float32)
    WL3 = np.zeros((128, 64 * NPAIR), np.float32)
    bb = np.zeros((128, 65), np.float32)
    bb[0:64, 0] = b3
    for t in range(NPAIR):
        a, b = 2 * t, 2 * t + 1
        c0 = 128 * t
        WL0[0:32, c0 : c0 + 64] = W0[a]
        WL0[32, c0 : c0 + 64] = b0[a]
        WL0[33:65, c0 + 64 : c0 + 128] = W0[b]
        WL0[65, c0 + 64 : c0 + 128] = b0[b]
        WL1[0:64, c0 : c0 + 64] = W1[a]
        WL1[64:128, c0 + 64 : c0 + 128] = W1[b]
        WL2[0:64, c0 : c0 + 64] = W2[a]
        WL2[64:128, c0 + 64 : c0 + 128] = W2[b]
        WL3[0:64, 64 * t + a] = W3[a]
        WL3[64:128, 64 * t + b] = W3[b]
        bb[0:64, 1 + t] = b1[a]
        bb[64:128, 1 + t] = b1[b]
        bb[0:64, 33 + t] = b2[a]
        bb[64:128, 33 + t] = b2[b]
    if MM_DT == "fp16":
        cast = lambda a: a.astype(np.float16).view(np.uint16)
    else:
        cast = lambda a: a
    return cast(WL0), cast(WL1), cast(WL2), cast(WL3), bb


def _prep_x(x):
    """Per-core [128, BL] tiles: x^T twice (rows 0:32 / 33:65) + ones rows."""
    xT = np.ascontiguousarray(np.asarray(x, np.float32).T)  # [I, B]
    tiles = []
    for core in range(NCORES):
        sl = xT[:, core * BL : (core + 1) * BL]
        t = np.zeros((66, BL), np.float32)
        t[0:32] = sl
        t[32] = 1.0
        t[33:65] = sl
        t[65] = 1.0
        tiles.append(t.astype(np.float16).view(np.uint16)
                     if MM_DT == "fp16" else t)
    return tiles


def kernel(x, W0, b0, W1, b1, W2, b2, W3, b3):
    from concourse.bass_utils import run_bass_kernel_spmd

    x, W0, b0, W1, b1, W2, b2, W3, b3 = (
        np.asarray(a, np.float32) for a in (x, W0, b0, W1, b1, W2, b2, W3, b3)
    )
    with_bias = bool(np.any(b1) or np.any(b2))
    key = ("nc", with_bias, MM_DT)
    if key not in _CACHE:
        _CACHE[key] = _build(with_bias, mm_dt=MM_DT)
    nc = _CACHE[key]

    WL0, WL1, WL2, WL3, bb = _prep_weights(W0, b0, W1, b1, W2, b2, W3, b3)
    xts = _prep_x(x)
    in_maps = [
        {"x2": xts[core], "wl0": WL0, "wl1": WL1, "wl2": WL2, "wl3": WL3, "bb": bb}
        for core in range(NCORES)
    ]
    res = run_bass_kernel_spmd(nc, in_maps, core_ids=list(range(NCORES)))
    out = np.concatenate(
        [r["outT"].T for r in res.results], axis=0
    )
    return np.ascontiguousarray(out, np.float32)


if __name__ == "__main__":
    rng = np.random.default_rng(0)
    inputs = {
        "x": rng.standard_normal((B, I), np.float32),
        "W0": rng.standard_normal((O, I, H), np.float32) / np.sqrt(I),
        "b0": np.zeros((O, H), np.float32),
        "W1": rng.standard_normal((O, H, H), np.float32) / np.sqrt(H),
        "b1": np.zeros((O, H), np.float32),
        "W2": rng.standard_normal((O, H, H), np.float32) / np.sqrt(H),
        "b2": np.zeros((O, H), np.float32),
        "W3": rng.standard_normal((O, H), np.float32) / np.sqrt(H),
        "b3": np.zeros((O,), np.float32),
    }
    out = kernel(**inputs)
    print(out.shape, out.dtype, float(np.abs(out).mean()))



# revision 4
# speedup vs baseline: 4.7434x; 4.7434x over previous
"""Trainium2 Bass kernel for nn_DividedModel (64 independent MLP towers).

Math (per tower o of O=64):
    h0 = relu(x @ W0[o] + b0[o])         x: [B, 32], W0[o]: [32, 64]
    h1 = relu(h0 @ W1[o] + b1[o])        W1[o]: [64, 64]
    h2 = relu(h1 @ W2[o] + b2[o])        W2[o]: [64, 64]
    out[:, o] = h2 @ W3[o] + b3[o]       W3[o]: [64]

Strategy (v2, PE-array tiling):
  - Data-parallel: batch B=16384 sharded 8 ways (2048 rows/core), params
    replicated; no collectives. Activations kept transposed ([h, batch]).
  - PE tiling (tile_position): L0/L1/L2 run in 64x64 mode - 4 concurrent
    matmuls per 512-cycle slot, one tower per 64x64 tile, ~100% PE
    utilization (the old kernel's block-diagonal pairs got 50%).
  - L3 runs in 128x32 column-tiling mode: 4 concurrent accumulation
    chains, each matmul contributing 2 towers' dot products.
  - All matmul operands fp16 (1 cycle/row; shipped as uint16 through the
    PJRT bridge and bitcast on-chip). PSUM stays fp32 (TRN2 requirement).
  - Evacuation (the real bottleneck, ~1 fp32/lane/cycle from PSUM on each
    of ScalarE/VectorE): one [128, 1024] tensor_scalar(max) op per slot,
    nc.any-routed so the Tile scheduler keeps both engines saturated.
  - Biases: all applied in the evacuation op via per-partition bias
    columns (nonzero-bias build splits each evac into two [128, 512] ops);
    b3 added during the final L3 PSUM copy.

Roofline: evac = 3 layers x 65536 cols / (0.96+1.2 GHz) ~ 95-105 us;
PE ~ 50 us (hidden). Old kernel: 150 us (PE-bound at 50% utilization).
"""

import numpy as np

B, I, O, H = 16384, 32, 64, 64
NCORES = 8
BL = B // NCORES      # 2048 batch rows per core
NB = 512              # batch columns per matmul (one fp32 PSUM bank)
CHUNKS = BL // NB     # 4
NT2 = O // 4          # 16 slots of 4 towers per (chunk, layer)
MM_DT = "fp16"        # matmul operand dtype: fp16 | f32r

_CACHE = {}


def _row_of_tower():
    """Map tower id -> partition row of the final L3 PSUM bank.

    L3 matmul j (= 4*p + cq, p in [0,8), cq in [0,4)) reads h2 tile t2=j//2
    bank b=j%2 (towers 4*t2+2b, 4*t2+2b+1) and writes PSUM partitions
    32*cq + 2p (+1)."""
    rows = np.zeros(O, np.int64)
    for j in range(32):
        cq, p = j % 4, j // 4
        t2, b = j // 2, j % 2
        rows[4 * t2 + 2 * b] = 32 * cq + 2 * p
        rows[4 * t2 + 2 * b + 1] = 32 * cq + 2 * p + 1
    return rows


def _build(with_bias: bool = False, reps: int = 1, mm_dt: str = MM_DT):
    import concourse.mybir as mybir
    import concourse.tile as tile
    from concourse import bacc

    f32 = mybir.dt.float32
    fp16_via_u16 = mm_dt == "fp16"
    mdt = mybir.dt.uint16 if fp16_via_u16 else mybir.dt.float32r

    def R(ap):
        # fp16 tensors are declared uint16 end-to-end (the axon PJRT
        # bridge rejects F16 transfers); bitcast to f16 at each use.
        return ap.bitcast(mybir.dt.float16) if fp16_via_u16 else ap

    add_op = mybir.AluOpType.add
    max_op = mybir.AluOpType.max

    nc = bacc.Bacc(None, target_bir_lowering=False, debug=False)

    x2_d = nc.dram_tensor("x2", [128, BL], mdt, kind="ExternalInput")
    wl0_d = nc.dram_tensor("wl0", [128, 128 * NT2], mdt, kind="ExternalInput")
    wl1_d = nc.dram_tensor("wl1", [128, 128 * NT2], mdt, kind="ExternalInput")
    wl2_d = nc.dram_tensor("wl2", [128, 128 * NT2], mdt, kind="ExternalInput")
    wl3_d = nc.dram_tensor("wl3", [128, 512], mdt, kind="ExternalInput")
    bb_d = nc.dram_tensor("bb", [128, 97], f32, kind="ExternalInput")
    outT_d = nc.dram_tensor("outT", [128, BL], f32, kind="ExternalOutput")

    hbufs = 48 if fp16_via_u16 else 34

    with tile.TileContext(nc) as tc:
        with (
            tc.tile_pool(name="w", bufs=1) as wpool,
            tc.tile_pool(name="h", bufs=hbufs) as hpool,
            tc.tile_pool(name="ot", bufs=3) as opool,
            tc.tile_pool(name="pp", bufs=3, space="PSUM") as ppool,
            tc.tile_pool(name="l3", bufs=2, space="PSUM") as l3pool,
        ):
            x2_s = wpool.tile([128, BL], mdt, tag="x2")
            wl0_s = wpool.tile([128, 128 * NT2], mdt, tag="wl0")
            wl1_s = wpool.tile([128, 128 * NT2], mdt, tag="wl1")
            wl2_s = wpool.tile([128, 128 * NT2], mdt, tag="wl2")
            wl3_s = wpool.tile([128, 512], mdt, tag="wl3")
            bb_s = wpool.tile([128, 97], f32, tag="bb")

            # Input loads split across the two DMA-capable queues that do
            # not occupy ScalarE/VectorE, ordered by first use.
            q = 64 * NT2
            nc.sync.dma_start(bb_s[:], bb_d[:])
            nc.sync.dma_start(x2_s[:], x2_d[:])
            nc.gpsimd.dma_start(wl0_s[:, :q], wl0_d[:, :q])
            nc.sync.dma_start(wl0_s[:, q:], wl0_d[:, q:])
            nc.gpsimd.dma_start(wl1_s[:, :q], wl1_d[:, :q])
            nc.sync.dma_start(wl1_s[:, q:], wl1_d[:, q:])
            nc.gpsimd.dma_start(wl2_s[:, :q], wl2_d[:, :q])
            nc.sync.dma_start(wl2_s[:, q:], wl2_d[:, q:])
            nc.gpsimd.dma_start(wl3_s[:], wl3_d[:])

            def evac(dst, pp, layer, t2):
                """dst[SBUF fp16] = relu(pp[PSUM fp32] + bias).

                nc.any: the Tile scheduler routes each op to whichever of
                ScalarE/VectorE is free."""
                if not with_bias:
                    nc.any.tensor_scalar_max(R(dst[:]), pp[:], 0.0)
                else:
                    for b in range(2):
                        col = 1 + 32 * layer + 2 * t2 + b
                        bias_ap = bb_s[:, col : col + 1]
                        nc.any.tensor_scalar(
                            R(dst[:, NB * b : NB * (b + 1)]),
                            pp[:, NB * b : NB * (b + 1)],
                            bias_ap, 0.0, add_op, max_op,
                        )

            def mm4(pp, wl, rhs_of, t2):
                """One 64x64-tiling slot: 4 concurrent one-tower matmuls.

                Tile (r, c): lhsT = wl[r:r+64, 128*t2 + c ...], rhs from
                partition half r, output -> psum[c:c+64, bank r//64]."""
                for r in (0, 64):
                    bk = (r // 64) * NB
                    for c in (0, 64):
                        lhsT = wl[r : r + 64, 128 * t2 + c : 128 * t2 + c + 64]
                        nc.tensor.matmul(
                            pp[c : c + 64, bk : bk + NB],
                            R(lhsT),
                            R(rhs_of(r, c)),
                            start=True,
                            stop=True,
                            tile_position=(r, c),
                        )

            for _rep in range(reps):
                for ch in range(CHUNKS):
                    cs = NB * ch
                    # ---- L0: h0 = relu(x W0 + b0), tile t2 banks hold
                    # pairs (4t2, 4t2+1), (4t2+2, 4t2+3)
                    h0 = []
                    for t2 in range(NT2):
                        pp = ppool.tile([128, 2 * NB], f32, tag="pp", name="pp0")
                        mm4(pp, wl0_s, lambda r, c: x2_s[r : r + 64, cs : cs + NB], t2)
                        dst = hpool.tile([128, 2 * NB], mdt, tag="h", name="h0")
                        evac(dst, pp, 0, t2)
                        h0.append(dst)
                    # ---- L1: consumes h0 tile t2 entirely; output banks
                    # hold pairs (4t2, 4t2+2), (4t2+1, 4t2+3)
                    h1 = []
                    for t2 in range(NT2):
                        pp = ppool.tile([128, 2 * NB], f32, tag="pp", name="pp1")
                        src = h0[t2]

                        def rhs1(r, c, src=src):
                            # partition half r, column bank c//64; which
                            # tower that is is encoded in the wl1 layout
                            b = c // 64
                            return src[r : r + 64, NB * b : NB * (b + 1)]

                        mm4(pp, wl1_s, rhs1, t2)
                        dst = hpool.tile([128, 2 * NB], mdt, tag="h", name="h1")
                        evac(dst, pp, 1, t2)
                        h1.append(dst)
                    # ---- L2: output banks restore pairs (4t2, 4t2+1),
                    # (4t2+2, 4t2+3)
                    h2 = []
                    for t2 in range(NT2):
                        pp = ppool.tile([128, 2 * NB], f32, tag="pp", name="pp2")
                        src = h1[t2]

                        def rhs2(r, c, src=src):
                            b = c // 64
                            return src[r : r + 64, NB * b : NB * (b + 1)]

                        mm4(pp, wl2_s, rhs2, t2)
                        dst = hpool.tile([128, 2 * NB], mdt, tag="h", name="h2")
                        evac(dst, pp, 2, t2)
                        h2.append(dst)
                    # ---- L3: 4 concurrent column-tiled chains (128x32
                    # mode); chain cq accumulates 8 matmuls, each filling
                    # psum rows 32cq+2p, 32cq+2p+1 of a [16, 512] slice.
                    l3p = l3pool.tile([128, NB], f32, tag="l3", name="l3p")
                    for cq in range(4):
                        for p in range(8):
                            j = 4 * p + cq
                            t2, b = j // 2, j % 2
                            nc.tensor.matmul(
                                l3p[32 * cq : 32 * cq + 16, :],
                                R(wl3_s[:, 16 * j : 16 * (j + 1)]),
                                R(h2[t2][:, NB * b : NB * (b + 1)]),
                                start=(p == 0),
                                stop=(p == 7),
                                tile_position=(0, 32 * cq),
                            )
                    out_sb = opool.tile([128, NB], f32, tag="ot")
                    nc.any.tensor_scalar(
                        out_sb[:], l3p[:], bb_s[:, 0:1], None, add_op
                    )
                    nc.sync.dma_start(outT_d[:, cs : cs + NB], out_sb[:])

    nc.compile()
    return nc


def _prep_weights(W0, b0, W1, b1, W2, b2, W3, b3):
    WL0 = np.zeros((128, 128 * NT2), np.float32)
    WL1 = np.zeros((128, 128 * NT2), np.float32)
    WL2 = np.zeros((128, 128 * NT2), np.float32)
    WL3 = np.zeros((128, 512), np.float32)
    bb = np.zeros((128, 97), np.float32)
    rows = _row_of_tower()
    for o in range(O):
        bb[rows[o], 0] = b3[o]
    for t2 in range(NT2):
        c0 = 128 * t2
        tw = [4 * t2, 4 * t2 + 1, 4 * t2 + 2, 4 * t2 + 3]
        # L0: tile (r, c) -> tower index 2*(r//64) + (c//64)
        WL0[0:32, c0 : c0 + 64] = W0[tw[0]]
        WL0[0:32, c0 + 64 : c0 + 128] = W0[tw[1]]
        WL0[64:96, c0 : c0 + 64] = W0[tw[2]]
        WL0[64:96, c0 + 64 : c0 + 128] = W0[tw[3]]
        # L1: lhsT at parts r holds the tower whose h0 lives at parts r:
        # parts 0-63: towers 4t2 (->c=0), 4t2+2 (->c=64); parts 64-127:
        # towers 4t2+1 (->c=0), 4t2+3 (->c=64)
        WL1[0:64, c0 : c0 + 64] = W1[tw[0]]
        WL1[0:64, c0 + 64 : c0 + 128] = W1[tw[2]]
        WL1[64:128, c0 : c0 + 64] = W1[tw[1]]
        WL1[64:128, c0 + 64 : c0 + 128] = W1[tw[3]]
        # L2: h1 layout: lo half = 4t2 (b0), 4t2+1 (b1); hi = 4t2+2, 4t2+3
        WL2[0:64, c0 : c0 + 64] = W2[tw[0]]
        WL2[0:64, c0 + 64 : c0 + 128] = W2[tw[1]]
        WL2[64:128, c0 : c0 + 64] = W2[tw[2]]
        WL2[64:128, c0 + 64 : c0 + 128] = W2[tw[3]]
        # biases, per (layer, bank) pair columns
        for bk in range(2):
            # h0 banks: (4t2, 4t2+1), (4t2+2, 4t2+3)
            lo, hi = tw[2 * bk], tw[2 * bk + 1]
            bb[0:64, 1 + 2 * t2 + bk] = b0[lo]
            bb[64:128, 1 + 2 * t2 + bk] = b0[hi]
            # h1 banks: (4t2, 4t2+2), (4t2+1, 4t2+3)
            lo, hi = tw[bk], tw[bk + 2]
            bb[0:64, 33 + 2 * t2 + bk] = b1[lo]
            bb[64:128, 33 + 2 * t2 + bk] = b1[hi]
            # h2 banks: (4t2, 4t2+1), (4t2+2, 4t2+3)
            lo, hi = tw[2 * bk], tw[2 * bk + 1]
            bb[0:64, 65 + 2 * t2 + bk] = b2[lo]
            bb[64:128, 65 + 2 * t2 + bk] = b2[hi]
    for j in range(32):
        cq, p = j % 4, j // 4
        t2, b = j // 2, j % 2
        lo, hi = 4 * t2 + 2 * b, 4 * t2 + 2 * b + 1
        WL3[0:64, 16 * j + 2 * p] = W3[lo]
        WL3[64:128, 16 * j + 2 * p + 1] = W3[hi]
    if MM_DT == "fp16":
        cast = lambda a: a.astype(np.float16).view(np.uint16)
    else:
        cast = lambda a: a
    return cast(WL0), cast(WL1), cast(WL2), cast(WL3), bb


def _prep_x(x):
    """Per-core [128, BL] tiles: x^T replicated on all four 32-row groups."""
    xT = np.ascontiguousarray(np.asarray(x, np.float32).T)  # [I, B]
    tiles = []
    for core in range(NCORES):
        sl = xT[:, core * BL : (core + 1) * BL]
        t = np.empty((128, BL), np.float32)
        for r in range(4):
            t[32 * r : 32 * (r + 1)] = sl
        tiles.append(t.astype(np.float16).view(np.uint16)
                     if MM_DT == "fp16" else t)
    return tiles


def kernel(x, W0, b0, W1, b1, W2, b2, W3, b3):
    from concourse.bass_utils import run_bass_kernel_spmd

    x, W0, b0, W1, b1, W2, b2, W3, b3 = (
        np.asarray(a, np.float32) for a in (x, W0, b0, W1, b1, W2, b2, W3, b3)
    )
    with_bias = bool(np.any(b0) or np.any(b1) or np.any(b2))
    key = ("nc", with_bias, MM_DT)
    if key not in _CACHE:
        _CACHE[key] = _build(with_bias, mm_dt=MM_DT)
    nc = _CACHE[key]

    WL0, WL1, WL2, WL3, bb = _prep_weights(W0, b0, W1, b1, W2, b2, W3, b3)
    xts = _prep_x(x)
    in_maps = [
        {"x2": xts[core], "wl0": WL0, "wl1": WL1, "wl2": WL2, "wl3": WL3, "bb": bb}
        for core in range(NCORES)
    ]
    res = run_bass_kernel_spmd(nc, in_maps, core_ids=list(range(NCORES)))
    rows = _row_of_tower()
    out = np.concatenate(
        [r["outT"][rows, :].T for r in res.results], axis=0
    )
    return np.ascontiguousarray(out, np.float32)


if __name__ == "__main__":
    rng = np.random.default_rng(0)
    inputs = {
        "x": rng.standard_normal((B, I), np.float32),
        "W0": rng.standard_normal((O, I, H), np.float32) / np.sqrt(I),
        "b0": np.zeros((O, H), np.float32),
        "W1": rng.standard_normal((O, H, H), np.float32) / np.sqrt(H),
        "b1": np.zeros((O, H), np.float32),
        "W2": rng.standard_normal((O, H, H), np.float32) / np.sqrt(H),
        "b2": np.zeros((O, H), np.float32),
        "W3": rng.standard_normal((O, H), np.float32) / np.sqrt(H),
        "b3": np.zeros((O,), np.float32),
    }
    out = kernel(**inputs)
    print(out.shape, out.dtype, float(np.abs(out).mean()))
